# revision 3
# baseline (speedup 1.0000x reference)
"""Trainium2 kernel for nn_CrossAttMultiplexer.

Reference math:
    q = x_r @ WQ ; k = s_r @ WK ; v = s_r @ WV      (per-pixel, c=96 "tokens", feat dim 1)
    scores[n,i,j] = (q.k)/sqrt(d) = g * x[n,i] * s[n,j]   with g = (WQ.WK)/sqrt(d)
    alpha = softmax_j(scores)
    out[n,i] = v[n,i] * sum_j alpha[n,i,j] = v[n,i] * 1 = s[n,i] * WV[0,0]

The softmax rows sum to exactly 1 and v broadcasts over the summed axis, so the
whole module collapses to a single scalar multiply: out = s * WV[0,0].
(Verified vs the fp32 jax reference: max abs err ~8e-8.)

Sharding: pure data parallel. The pseudo-batch N = 4*64*64 = 16384 rows of 96
floats is split into 8 contiguous shards of 2048 rows; each core sees its shard
reinterpreted as a [128, 1536] SBUF tile. Weights fold into an immediate scalar
baked into the DVE instruction.

Implementation: raw Bass (no TileContext — its kernel-tail Drain exceeds the
walrus sync-wait limit on this compile path). Pipeline over column tiles:
  sync engine  : HWDGE loads  s_shard tile -> SBUF   (ring qSPDynamicHW)
  vector (DVE) : tensor_scalar_mul by WV            (fp32 2x mode)
  scalar (ACT) : HWDGE stores SBUF tile -> out       (ring qActDynamicHW)
Separate rings let the in and out streams overlap.
"""

import numpy as np

# Full-problem constants (hardcoded per harness contract).
B, H, W, C = 4, 64, 64, 96
N_CORES = 8
P = 128                                # SBUF partitions
F = (B * H * W * C) // (N_CORES * P)   # 1536 floats per partition per core

_PROG_CACHE: dict = {}


def _build_program(wv: float, n_tiles: int = 4):
    import concourse.bass as bass
    from concourse import mybir

    f32 = mybir.dt.float32
    ts = F // n_tiles
    assert ts * n_tiles == F

    nc = bass.Bass()
    s_in = nc.declare_dram_parameter("s_shard", [P, F], f32, isOutput=False)
    out_ext = nc.declare_dram_parameter("out", [P, F], f32, isOutput=True)

    with (
        nc.Block() as block,
        nc.semaphore("in_sem") as in_sem,
        nc.semaphore("v_sem") as v_sem,
        nc.semaphore("out_sem") as out_sem,
        nc.sbuf_tensor("in_buf", [P, F], f32) as in_buf,
        nc.sbuf_tensor("out_buf", [P, F], f32) as out_buf,
    ):

        @block.sync
        def _(sync):
            for i in range(n_tiles):
                sync.dma_start(
                    out=in_buf[:, i * ts : (i + 1) * ts],
                    in_=s_in[:, i * ts : (i + 1) * ts],
                ).then_inc(in_sem, 16)

        @block.vector
        def _(vector):
            for i in range(n_tiles):
                vector.wait_ge(in_sem, 16 * (i + 1))
                vector.tensor_scalar_mul(
                    out_buf[:, i * ts : (i + 1) * ts],
                    in_buf[:, i * ts : (i + 1) * ts],
                    wv,
                ).then_inc(v_sem, 1)

        @block.scalar
        def _(scalar):
            for i in range(n_tiles):
                scalar.wait_ge(v_sem, i + 1)
                scalar.dma_start(
                    out=out_ext[:, i * ts : (i + 1) * ts],
                    in_=out_buf[:, i * ts : (i + 1) * ts],
                ).then_inc(out_sem, 16)
            scalar.wait_ge(out_sem, 16 * n_tiles)

    return nc


def _get_program(wv: float, n_tiles: int = 4):
    key = (np.float32(wv).tobytes(), n_tiles)
    if key not in _PROG_CACHE:
        _PROG_CACHE[key] = _build_program(wv, n_tiles)
    return _PROG_CACHE[key]


def _run(x, s, WQ, WK, WV, trace: bool = False, n_tiles: int = 4):
    from concourse.bass_utils import run_bass_kernel_spmd

    s = np.ascontiguousarray(np.asarray(s, dtype=np.float32))
    wv = float(np.asarray(WV, dtype=np.float32).reshape(-1)[0])

    shards = s.reshape(N_CORES, P, F)
    in_maps = [{"s_shard": shards[i]} for i in range(N_CORES)]

    nc = _get_program(wv, n_tiles)
    res = run_bass_kernel_spmd(nc, in_maps, list(range(N_CORES)), trace=trace)
    out = np.stack([np.asarray(res.results[i]["out"]) for i in range(N_CORES)])
    return out.reshape(B, H, W, C).astype(np.float32, copy=False), res


def kernel(x, s, WQ, WK, WV):
    out, _ = _run(x, s, WQ, WK, WV)
    return out
